# revision 41
# baseline (speedup 1.0000x reference)
"""LIF spike kernel (T=4 scan with threshold reset) on 8 TRN2 NeuronCores.

Recurrence per element (tau=1, thresh=1):
    s_t     = m_{t-1} + x_t
    spike_t = (s_t > 1)           -> output
    m_t     = s_t * (s_t <= 1)    -> threshold reset

Sharding: pure data-parallel over the batch axis (dim 1, 64 -> 8 per core).

Roofline: the 16 SDMA engines process 4KB packets back-to-back at 158 ns
(~26 GB/s each, ~414 GB/s/core); that per-engine packet rate is the wall.
All f32 loads ride the SP HWDGE ring (a single HWDGE queue streams at
~97% of engine rate; SWDGE packets are ~30% slower) in 4KB-per-partition
pieces (8KB runs stall ~430 ns between packets). Spikes are 0/1 and are
stored as int8 sign values (4.2 MB instead of 16.8) on the ACT HWDGE
ring; the host maps (stored == -1) -> f32.

Compute (per [128,2048] chunk, 4 chunks/core, bufs=4):
  DVE  - one runtime-registered custom op per step fuses reset+add:
            s_{t+1} = s_t * (s_t <= 1) + x_{t+1}
         (t=0 folds in for free since s_0 = x_0), written in-place into
         the x_{t+1} tile. 3 ops/chunk instead of 6 -> ~29 us/core,
         bit-exact (verified on HW).
  ACT  - spike_t = Sign(1 - s_t) -> int8 via scale=-1/bias=1 (the 1.0
         const AP is pre-registered; (N+352)/1.2 ns per op; exact: 1-s
         is Sterbenz-exact near 1, so sign(1-s)==-1 iff s>1).
  Pool - NO compute and NO DMA (its TS/TT ops are 13x slower than
         modeled, their SBUF traffic slows DVE 3.7x, SWDGE packets are
         slow, and CCE-accum loads RMW at only ~147 GB/s - all measured).
dma_start dispatch can block 1-3 us on ring credit, so loads never
dispatch from a compute engine; stores dispatch on ACT only after its
own signs, which never blocks.
"""

import numpy as np

import concourse.bacc as bacc
import concourse.mybir as mybir
import concourse.tile as tile
from concourse import bass_utils
from concourse.dve_ops import (
    CUSTOM_DVE_SPECS,
    OPS,
    _CUSTOM_DVE_ROW_BASE,
    _SUB_OPCODE_FOR_NAME,
    DveOp,
    has_src1,
)
from concourse.dve_spec import One, Spec, Src0, Src1, lower
from concourse.dve_uop import DveOpSpec

T = 4
B_FULL = 64
C, H, W = 128, 32, 32
N_CORES = 8
B_LOC = B_FULL // N_CORES            # 8
N = B_LOC * C * H * W                # 1048576 elements per core per timestep
P = 128                              # SBUF partitions

_SIGN = mybir.ActivationFunctionType.Sign

F = 2048
BUFS = 4


def _register_lif_op():
    """Register the fused reset+add DVE op (documented custom-op path:
    append a Spec to dve_ops.OPS; the per-NEFF DVE table is generated from
    it at compile time). Idempotent for repeated imports."""
    name = "LIF_RESET_ADD_ANT"
    for op in OPS:
        if op.name == name:
            return op
    spec = Spec(
        body=Src0 * (Src0 <= One) + Src1,
        reference=lambda in0, in1, s0, s1, imm2: (
            in0 * (in0 <= 1.0) + in1
        ).astype(np.float32),
    )
    row = _CUSTOM_DVE_ROW_BASE + len(OPS)
    _SUB_OPCODE_FOR_NAME[name] = row
    shas = {
        ver: DveOpSpec(
            name=name, opcode=row, uops=lower(spec, ver=ver), rd1_en=has_src1(spec)
        ).sha(ver)
        for ver in ("v3", "v4")
    }
    op = DveOp(name, spec, subdim=False, uops_sha=shas)
    OPS.append(op)
    CUSTOM_DVE_SPECS[name] = spec
    return op


_LIF_OP = _register_lif_op()

_nc_cache = None


def _build(F=F, bufs=BUFS):
    nchunk = N // (P * F)
    nc = bacc.Bacc(
        "TRN2",
        target_bir_lowering=False,
        debug=False,
        enable_asserts=False,
    )
    x_d = nc.dram_tensor("x", [T, N], mybir.dt.float32, kind="ExternalInput").ap()
    y_d = nc.dram_tensor(
        "y", [nchunk, P, T * F], mybir.dt.int8, kind="ExternalOutput"
    ).ap()
    # [t, n, p, f] view of the flat [T, N] input, at 4KB-per-partition DMA
    # granularity: 4KB packets stream back-to-back at the ~27 GB/s port
    # rate; 8KB packets stall ~430ns between packets (measured), so every
    # dma_start moves a [P, 1024] f32 (or [P, 4096] int8) piece
    FL = 1024                       # f32 elems per partition per DMA piece
    sub = F // FL
    xv = x_d.rearrange("t (n p f) -> t n p f", p=P, f=F)

    with tile.TileContext(nc) as tc:
        with (
            tc.tile_pool(name="spk", bufs=bufs) as spp,
            tc.tile_pool(name="wrk", bufs=bufs) as wkp,
            tc.tile_pool(name="last", bufs=1) as lap,
        ):
            for j in range(nchunk):
                def wtile(tag):
                    return wkp.tile(
                        [P, F], mybir.dt.float32, tag=tag, name=f"{tag}_{j}"
                    )[:]

                spall = spp.tile([P, T * F], mybir.dt.int8, tag="s", name=f"s_{j}")
                sp = [spall[:, t * F : (t + 1) * F] for t in range(T)]

                last = j == nchunk - 1
                xt = [wtile(f"x{t}") for t in range(T - 1)]
                if last:
                    x3h = [
                        lap.tile(
                            [P, FL], mybir.dt.float32, tag=f"h{k}", name=f"x3h{k}"
                        )[:]
                        for k in range(sub)
                    ]
                else:
                    xt.append(wtile(f"x{T - 1}"))
                # ALL loads on the SP HWDGE ring: a single HWDGE queue
                # streams the 16 SDMA engines at ~97% of port rate
                # (measured), SWDGE packets are ~30% slower, and dma_start
                # dispatch can block ~1-3us on ring credit so it must not
                # sit on a compute engine
                for t in range(T):
                    for k in range(sub):
                        fs = slice(k * FL, (k + 1) * FL)
                        if t == T - 1 and last:
                            nc.sync.dma_start(x3h[k], xv[t, j][:, fs])
                        else:
                            nc.sync.dma_start(xt[t][:, fs], xv[t, j][:, fs])

                # spike_t = (stored == -1): ACT computes sign(1 - s_t)
                # (scale=-1 with the pre-registered bias=1.0 const; exact,
                # 1-s is Sterbenz-exact near 1 so sign(1-s)==-1 iff s>1)
                nc.scalar.activation(sp[0], xt[0], _SIGN, bias=1.0, scale=-1.0)
                s = xt[0]
                for t in range(1, T - 1):
                    # s_t = s_{t-1}*(s_{t-1}<=1) + x_t, in-place into the
                    # x_t tile (dead after this op) -- keeps the pool at 4
                    # f32 tiles so bufs=4 fits in SBUF
                    nc.vector._custom_dve(_LIF_OP, out=xt[t], in0=s, in1=xt[t])
                    nc.scalar.activation(sp[t], xt[t], _SIGN, bias=1.0, scale=-1.0)
                    s = xt[t]
                if last:
                    # final chunk: t=3 runs on half tiles so the chain
                    # after the very last load packet is half-length
                    for k in range(sub):
                        fs = slice(k * FL, (k + 1) * FL)
                        nc.vector._custom_dve(
                            _LIF_OP, out=x3h[k], in0=s[:, fs], in1=x3h[k]
                        )
                        nc.scalar.activation(
                            sp[T - 1][:, fs], x3h[k], _SIGN, bias=1.0, scale=-1.0
                        )
                else:
                    nc.vector._custom_dve(
                        _LIF_OP, out=xt[T - 1], in0=s, in1=xt[T - 1]
                    )
                    nc.scalar.activation(
                        sp[T - 1], xt[T - 1], _SIGN, bias=1.0, scale=-1.0
                    )

                # both stores at chunk end on the ACT ring, 4KB/partition
                # pieces. A/B'd twice: eager mid-chunk stores spread store
                # packets through the load phase and cost ~15% DMA
                # efficiency; t3-spike-on-DVE (split spike tiles) was also
                # consistently ~5us slower than this shape.
                for k in range(0, T * F, 4 * FL):
                    ks = slice(k, k + 4 * FL)
                    nc.scalar.dma_start(y_d[j][:, ks], spall[:, ks])

    nc.compile()
    return nc


def _get_nc():
    global _nc_cache
    if _nc_cache is None:
        _nc_cache = _build()
    return _nc_cache


def _run(x, **spmd_kwargs):
    x = np.asarray(x, dtype=np.float32)
    assert x.shape == (T, B_FULL, C, H, W), x.shape
    nchunk = N // (P * F)
    in_maps = [
        {
            "x": np.ascontiguousarray(
                x[:, c * B_LOC : (c + 1) * B_LOC]
            ).reshape(T, N)
        }
        for c in range(N_CORES)
    ]
    res = bass_utils.run_bass_kernel_spmd(
        _get_nc(), in_maps, core_ids=list(range(N_CORES)), **spmd_kwargs
    )
    out = np.empty((T, B_FULL, C, H, W), dtype=np.float32)
    for c in range(N_CORES):
        y = res.results[c]["y"]  # [nchunk, P, T*F] int8, sign(1-s) in {-1,0,1}
        spikes = (
            y.reshape(nchunk, P, T, F).transpose(2, 0, 1, 3).reshape(T, N) == -1
        )
        out[:, c * B_LOC : (c + 1) * B_LOC] = spikes.reshape(
            T, B_LOC, C, H, W
        )
    return out, res


def kernel(x):
    out, _ = _run(x)
    return out
